# revision 10
# baseline (speedup 1.0000x reference)
"""Trainium2 Bass kernel for DenseRoutingMaskLayer (MoE routing chunk-gather).

reference: route = argmax(routing_inputs, -1); out[b] = inputs[b].reshape(8, 512)[route[b]]

Pure data parallel across 8 NeuronCores (2048 rows each). Per core, raw-bacc
program:

  ACT : loads routing (laid out [32, 4, 16, 8]: partition p holds rows
        512k+16p+u, each a contiguous 512B run) and the index-weight tile;
        later stores odd gathered sub-blocks (2nd HWDGE ring)
  DVE : 6-op chain, relying on the engine's in-order pipe drain (no
        self-semaphores): reduce_max over routes, is_equal vs the max,
        multiply by wt = r + 8*row, reduce_add -> flat gather index
        idx(i) = 8*i + route(i) exactly in f32; convert f32->i32; then one
        32x32-block stream transpose of the (broadcast-doubled) low halves
        lands the wrapped+replicated int16 index layout [32, 128] directly
  POOL: loads the mlp Q7 library first (its ~9us fetch overlaps the whole
        index pipeline), then 8x {dma_gather(prepare_only) -> trigger_dma}.
        Decoupling descriptor generation from the DMA drain keeps the 16
        DMA engines fed back-to-back instead of stalling ~1.5us per
        sub-gather inside each blocking gather instruction.
  SP  : stores even sub-blocks (1st HWDGE ring)

The gathered row i lands at SBUF partition i%128, col i//128; stores use a
matching strided DRAM view (output y [128, 16, 512]; host transposes back).
Index tile partitions 32..127 are memset to 0 (the queue-0 gather ucode
only reads partitions 0..31).
"""

import sys

import numpy as np

try:
    import concourse  # noqa: F401
except ImportError:  # pragma: no cover
    sys.path.insert(0, "/opt/trn_rl_repo")

N_CORES = 8
B_FULL = 16384
D = 4096
ROUTES = 8
RW = D // ROUTES
B_SH = B_FULL // N_CORES  # 2048
NJ = B_SH // 128  # 16 output cols
# sub-gather sizes in 128-row cols: ramp up (amortize the ~1us fixed
# ucode cost per call) and back down (short store tail)
JSIZES = [1, 2, 3, 4, 3, 2, 1]
JBOUNDS = [sum(JSIZES[:i]) for i in range(len(JSIZES) + 1)]  # [0,1,3,6,10,13,15,16]
NG = len(JSIZES)
KB = 4  # k-blocks in the [32, KB, 16, 8] routing layout

_prog_cache = {}


def _build_program():
    import concourse.bacc as bacc
    import concourse.mybir as mybir
    from concourse.library_config import mlp
    from contextlib import ExitStack

    f32 = mybir.dt.float32
    i32 = mybir.dt.int32
    i16 = mybir.dt.int16
    Alu = mybir.AluOpType
    Axis = mybir.AxisListType

    nc = bacc.Bacc("TRN2", target_bir_lowering=False, debug=False, num_devices=N_CORES)
    x = nc.dram_tensor("x", [B_SH, D], f32, kind="ExternalInput")
    rt = nc.dram_tensor("rt", [B_SH, ROUTES], f32, kind="ExternalInput")
    wt = nc.dram_tensor("wt", [32, KB, 16, ROUTES], f32, kind="ExternalInput")
    # partition-major output: y[p, j, :] holds row j*128+p; the host
    # transposes back. Keeps every store descriptor 4KB-contiguous.
    y = nc.dram_tensor("y", [128, NJ, RW], f32, kind="ExternalOutput")

    x_rows = x.ap().rearrange("b (r w) -> (b r) w", r=ROUTES)
    # row = 512k + 16p + u
    rt_n = rt.ap().rearrange("(k p u) r -> p k u r", k=KB, p=32, u=16)
    y_pjw = y.ap()

    with (
        ExitStack() as ctx,
        nc.sbuf_tensor("r_t", [32, KB, 16, ROUTES], f32) as r_t,
        nc.sbuf_tensor("wt_t", [32, KB, 16, ROUTES], f32) as wt_t,
        nc.sbuf_tensor("mx", [32, KB, 16], f32) as mx,
        nc.sbuf_tensor("eq", [32, KB, 16, ROUTES], f32) as eq,
        nc.sbuf_tensor("idf", [32, KB * 16], f32) as idf,
        nc.sbuf_tensor("idfi", [32, KB * 16], i32) as idfi,
        nc.sbuf_tensor("idx16", [128, 128], i16) as idx16,
        nc.sbuf_tensor("g_t", [128, NJ, RW], f32) as g_t,
        nc.Block(no_gpsimd_drain=True) as block,
    ):
        s_rt = ctx.enter_context(nc.semaphore("s_rt"))
        s_wt = ctx.enter_context(nc.semaphore("s_wt"))
        s_v = ctx.enter_context(nc.semaphore("s_v"))
        s_g = [ctx.enter_context(nc.semaphore(f"s_g{k}")) for k in range(NG)]
        s_y = ctx.enter_context(nc.semaphore("s_y"))

        @block.scalar
        def _(act):
            act.dma_start(r_t[:], rt_n).then_inc(s_rt, 16)
            act.dma_start(wt_t[:], wt.ap()).then_inc(s_wt, 16)
            for k in range(NG):
                js = slice(JBOUNDS[k], JBOUNDS[k + 1])
                act.wait_ge(s_g[k], 16)
                act.dma_start(y_pjw[0:64, js, :], g_t[0:64, js, :]).then_inc(s_y, 16)

        @block.vector
        def _(dve):
            dve.memset(idx16[:], 0)
            dve.wait_ge(s_rt, 16)
            dve.tensor_reduce(mx[:], r_t[:], Axis.X, Alu.max)
            dve.tensor_tensor(
                eq[:],
                r_t[:],
                mx[:].unsqueeze(3).broadcast_to([32, KB, 16, ROUTES]),
                Alu.is_equal,
            )
            dve.wait_ge(s_wt, 16)
            dve.tensor_tensor(eq[:], eq[:], wt_t[:], Alu.mult)
            dve.tensor_reduce(idf[:], eq[:], Axis.X, Alu.add)
            dve.tensor_copy(idfi[:], idf[:])
            # low int16 halves, doubled along a 0-stride dim, then a 32x32
            # block transpose: idx16[q, 32k+16d+u] = idx(16c + q%16).
            t_in = (
                idfi.ap()
                .bitcast(i16)
                .rearrange("q (k u two) -> q k u two", k=KB, u=16, two=2)[:, :, :, 0]
                .unsqueeze(2)
                .broadcast_to([32, KB, 2, 16])
            )
            t_out = idx16.ap()[0:32, :].rearrange("q (k d u) -> q k d u", k=KB, d=2, u=16)
            dve.transpose(t_out, t_in).then_inc(s_v, 1)

        @block.gpsimd
        def _(pool):
            pool.load_library(mlp)
            pool.wait_ge(s_v, 1)
            for k, (j0, j1) in enumerate(zip(JBOUNDS, JBOUNDS[1:])):
                js = slice(j0, j1)
                cs = slice(8 * j0, 8 * j1)
                rows = 128 * (j1 - j0)
                pool.dma_gather(
                    g_t[:, js, :],
                    x_rows,
                    idx16[:, cs],
                    rows,
                    rows,
                    RW,
                    single_packet=False,
                ).then_inc(s_g[k], 16)

        @block.sync
        def _(sp):
            for k in range(NG):
                js = slice(JBOUNDS[k], JBOUNDS[k + 1])
                sp.wait_ge(s_g[k], 16)
                sp.dma_start(y_pjw[64:128, js, :], g_t[64:128, js, :]).then_inc(
                    s_y, 16
                )
            sp.wait_ge(s_y, 32 * NG)

    nc.compile()
    return nc


def _get_program():
    if "p" not in _prog_cache:
        _prog_cache["p"] = _build_program()
    return _prog_cache["p"]


def _weights():
    # wt[p, k, u, r] = r + 8*row, row = 512k + 16p + u
    p = np.arange(32, dtype=np.float32)[:, None, None, None]
    k = np.arange(KB, dtype=np.float32)[None, :, None, None]
    u = np.arange(16, dtype=np.float32)[None, None, :, None]
    r = np.arange(ROUTES, dtype=np.float32)[None, None, None, :]
    return np.ascontiguousarray(r + 8.0 * (512.0 * k + 16.0 * p + u), dtype=np.float32)


def _in_maps(inputs, routing_inputs):
    wt = _weights()
    return [
        {
            "x": inputs[c * B_SH : (c + 1) * B_SH],
            "rt": routing_inputs[c * B_SH : (c + 1) * B_SH],
            "wt": wt,
        }
        for c in range(N_CORES)
    ]


def kernel(inputs: np.ndarray, routing_inputs: np.ndarray) -> np.ndarray:
    from concourse.bass_utils import run_bass_kernel_spmd

    inputs = np.ascontiguousarray(inputs, dtype=np.float32)
    routing_inputs = np.ascontiguousarray(routing_inputs, dtype=np.float32)
    nc = _get_program()
    in_maps = _in_maps(inputs, routing_inputs)
    res = None
    for attempt in range(3):
        try:
            res = run_bass_kernel_spmd(nc, in_maps, core_ids=list(range(N_CORES)))
            break
        except Exception:  # transient NRT_EXEC_UNIT_UNRECOVERABLE flakes
            if attempt == 2:
                raise
            import time

            time.sleep(2.0)
    return np.concatenate(
        [
            res.results[c]["y"].transpose(1, 0, 2).reshape(B_SH, RW)
            for c in range(N_CORES)
        ],
        axis=0,
    )


# revision 13
# speedup vs baseline: 1.0089x; 1.0089x over previous
"""Trainium2 Bass kernel for DenseRoutingMaskLayer (MoE routing chunk-gather).

reference: route = argmax(routing_inputs, -1); out[b] = inputs[b].reshape(8, 512)[route[b]]

Pure data parallel across 8 NeuronCores (2048 rows each). Per core, raw-bacc
program:

  ACT : loads routing (laid out [32, 4, 16, 8]: partition p holds rows
        512k+16p+u, each a contiguous 512B run) and the index-weight tile;
        later stores odd gathered sub-blocks (2nd HWDGE ring)
  DVE : 6-op chain, relying on the engine's in-order pipe drain (no
        self-semaphores): reduce_max over routes, is_equal vs the max,
        multiply by wt = r + 8*row, reduce_add -> flat gather index
        idx(i) = 8*i + route(i) exactly in f32; convert f32->i32; then one
        32x32-block stream transpose of the (broadcast-doubled) low halves
        lands the wrapped+replicated int16 index layout [32, 128] directly
  POOL: loads the mlp Q7 library first (its ~9us fetch overlaps the whole
        index pipeline), then 8x {dma_gather(prepare_only) -> trigger_dma}.
        Decoupling descriptor generation from the DMA drain keeps the 16
        DMA engines fed back-to-back instead of stalling ~1.5us per
        sub-gather inside each blocking gather instruction.
  SP  : stores even sub-blocks (1st HWDGE ring)

The gathered row i lands at SBUF partition i%128, col i//128; stores use a
matching strided DRAM view (output y [128, 16, 512]; host transposes back).
Index tile partitions 32..127 are memset to 0 (the queue-0 gather ucode
only reads partitions 0..31).
"""

import sys

import numpy as np

try:
    import concourse  # noqa: F401
except ImportError:  # pragma: no cover
    sys.path.insert(0, "/opt/trn_rl_repo")

N_CORES = 8
B_FULL = 16384
D = 4096
ROUTES = 8
RW = D // ROUTES
B_SH = B_FULL // N_CORES  # 2048
NJ = B_SH // 128  # 16 output cols
# sub-gather sizes in 128-row cols: ramp up (amortize the ~1us fixed
# ucode cost per call) and back down (short store tail)
JSIZES = [1, 2, 3, 4, 3, 2, 1]
JBOUNDS = [sum(JSIZES[:i]) for i in range(len(JSIZES) + 1)]  # [0,1,3,6,10,13,15,16]
NG = len(JSIZES)
KB = 4  # k-blocks in the [32, KB, 16, 8] routing layout

_prog_cache = {}


def _build_program():
    import concourse.bacc as bacc
    import concourse.mybir as mybir
    from concourse.library_config import mlp
    from contextlib import ExitStack

    f32 = mybir.dt.float32
    i32 = mybir.dt.int32
    i16 = mybir.dt.int16
    Alu = mybir.AluOpType
    Axis = mybir.AxisListType

    nc = bacc.Bacc("TRN2", target_bir_lowering=False, debug=False, num_devices=N_CORES)
    x = nc.dram_tensor("x", [B_SH, D], f32, kind="ExternalInput")
    rt = nc.dram_tensor("rt", [B_SH, ROUTES], f32, kind="ExternalInput")
    wt = nc.dram_tensor("wt", [32, KB, 16, ROUTES], f32, kind="ExternalInput")
    # partition-major output: y[p, j, :] holds row j*128+p; the host
    # transposes back. Keeps every store descriptor 4KB-contiguous.
    y = nc.dram_tensor("y", [128, NJ, RW], f32, kind="ExternalOutput")

    x_rows = x.ap().rearrange("b (r w) -> (b r) w", r=ROUTES)
    # row = 512k + 16p + u
    rt_n = rt.ap().rearrange("(k p u) r -> p k u r", k=KB, p=32, u=16)
    y_pjw = y.ap()

    with (
        ExitStack() as ctx,
        nc.sbuf_tensor("r_t", [32, KB, 16, ROUTES], f32) as r_t,
        nc.sbuf_tensor("wt_t", [32, KB, 16, ROUTES], f32) as wt_t,
        nc.sbuf_tensor("mx", [32, KB, 16], f32) as mx,
        nc.sbuf_tensor("eq", [32, KB, 16, ROUTES], f32) as eq,
        nc.sbuf_tensor("idf", [32, KB * 16], f32) as idf,
        nc.sbuf_tensor("idfi", [32, KB * 16], i32) as idfi,
        nc.sbuf_tensor("idx16", [128, 128], i16) as idx16,
        nc.sbuf_tensor("g_t", [128, NJ, RW], f32) as g_t,
        nc.Block(no_gpsimd_drain=True) as block,
    ):
        s_rt = ctx.enter_context(nc.semaphore("s_rt"))
        s_wt = ctx.enter_context(nc.semaphore("s_wt"))
        s_v = ctx.enter_context(nc.semaphore("s_v"))
        s_g = [ctx.enter_context(nc.semaphore(f"s_g{k}")) for k in range(NG)]
        s_y = ctx.enter_context(nc.semaphore("s_y"))

        @block.scalar
        def _(act):
            act.dma_start(r_t[:], rt_n).then_inc(s_rt, 16)
            act.dma_start(wt_t[:], wt.ap()).then_inc(s_wt, 16)
            for k in range(1, NG, 2):
                js = slice(JBOUNDS[k], JBOUNDS[k + 1])
                act.wait_ge(s_g[k], 16)
                act.dma_start(y_pjw[:, js, :], g_t[:, js, :]).then_inc(s_y, 16)

        @block.vector
        def _(dve):
            dve.memset(idx16[:], 0)
            dve.wait_ge(s_rt, 16)
            dve.tensor_reduce(mx[:], r_t[:], Axis.X, Alu.max)
            dve.tensor_tensor(
                eq[:],
                r_t[:],
                mx[:].unsqueeze(3).broadcast_to([32, KB, 16, ROUTES]),
                Alu.is_equal,
            )
            dve.wait_ge(s_wt, 16)
            dve.tensor_tensor(eq[:], eq[:], wt_t[:], Alu.mult)
            dve.tensor_reduce(idf[:], eq[:], Axis.X, Alu.add)
            dve.tensor_copy(idfi[:], idf[:])
            # low int16 halves, doubled along a 0-stride dim, then a 32x32
            # block transpose: idx16[q, 32k+16d+u] = idx(16c + q%16).
            t_in = (
                idfi.ap()
                .bitcast(i16)
                .rearrange("q (k u two) -> q k u two", k=KB, u=16, two=2)[:, :, :, 0]
                .unsqueeze(2)
                .broadcast_to([32, KB, 2, 16])
            )
            t_out = idx16.ap()[0:32, :].rearrange("q (k d u) -> q k d u", k=KB, d=2, u=16)
            dve.transpose(t_out, t_in).then_inc(s_v, 1)

        @block.gpsimd
        def _(pool):
            pool.load_library(mlp)
            pool.wait_ge(s_v, 1)
            for k, (j0, j1) in enumerate(zip(JBOUNDS, JBOUNDS[1:])):
                js = slice(j0, j1)
                cs = slice(8 * j0, 8 * j1)
                rows = 128 * (j1 - j0)
                pool.dma_gather(
                    g_t[:, js, :],
                    x_rows,
                    idx16[:, cs],
                    rows,
                    rows,
                    RW,
                    single_packet=True,
                ).then_inc(s_g[k], 16)

        @block.sync
        def _(sp):
            for k in range(0, NG, 2):
                js = slice(JBOUNDS[k], JBOUNDS[k + 1])
                sp.wait_ge(s_g[k], 16)
                sp.dma_start(y_pjw[:, js, :], g_t[:, js, :]).then_inc(s_y, 16)
            sp.wait_ge(s_y, 16 * NG)

    nc.compile()
    return nc


def _get_program():
    if "p" not in _prog_cache:
        _prog_cache["p"] = _build_program()
    return _prog_cache["p"]


def _weights():
    # wt[p, k, u, r] = r + 8*row, row = 512k + 16p + u
    p = np.arange(32, dtype=np.float32)[:, None, None, None]
    k = np.arange(KB, dtype=np.float32)[None, :, None, None]
    u = np.arange(16, dtype=np.float32)[None, None, :, None]
    r = np.arange(ROUTES, dtype=np.float32)[None, None, None, :]
    return np.ascontiguousarray(r + 8.0 * (512.0 * k + 16.0 * p + u), dtype=np.float32)


def _in_maps(inputs, routing_inputs):
    wt = _weights()
    return [
        {
            "x": inputs[c * B_SH : (c + 1) * B_SH],
            "rt": routing_inputs[c * B_SH : (c + 1) * B_SH],
            "wt": wt,
        }
        for c in range(N_CORES)
    ]


def kernel(inputs: np.ndarray, routing_inputs: np.ndarray) -> np.ndarray:
    from concourse.bass_utils import run_bass_kernel_spmd

    inputs = np.ascontiguousarray(inputs, dtype=np.float32)
    routing_inputs = np.ascontiguousarray(routing_inputs, dtype=np.float32)
    nc = _get_program()
    in_maps = _in_maps(inputs, routing_inputs)
    res = None
    for attempt in range(3):
        try:
            res = run_bass_kernel_spmd(nc, in_maps, core_ids=list(range(N_CORES)))
            break
        except Exception:  # transient NRT_EXEC_UNIT_UNRECOVERABLE flakes
            if attempt == 2:
                raise
            import time

            time.sleep(2.0)
    return np.concatenate(
        [
            res.results[c]["y"].transpose(1, 0, 2).reshape(B_SH, RW)
            for c in range(N_CORES)
        ],
        axis=0,
    )


# revision 15
# speedup vs baseline: 1.0098x; 1.0009x over previous
"""Trainium2 Bass kernel for DenseRoutingMaskLayer (MoE routing chunk-gather).

reference: route = argmax(routing_inputs, -1); out[b] = inputs[b].reshape(8, 512)[route[b]]

Pure data parallel across 8 NeuronCores (2048 rows each). Per core, raw-bacc
program:

  ACT : loads routing (laid out [32, 4, 16, 8]: partition p holds rows
        512k+16p+u, each a contiguous 512B run) and the index-weight tile;
        later stores odd gathered sub-blocks (2nd HWDGE ring)
  DVE : 6-op chain, relying on the engine's in-order pipe drain (no
        self-semaphores): reduce_max over routes, is_equal vs the max,
        multiply by wt = r + 8*row, reduce_add -> flat gather index
        idx(i) = 8*i + route(i) exactly in f32; convert f32->i32; then one
        32x32-block stream transpose of the (broadcast-doubled) low halves
        lands the wrapped+replicated int16 index layout [32, 128] directly
  POOL: loads the mlp Q7 library first (its ~9us fetch overlaps the whole
        index pipeline; the first gather ucode call stalls until the lib is
        resident), then 7 gpsimd.dma_gather calls sized [1,2,3,4,3,2,1]
        output cols (128 rows/col). Descriptor gen costs ~580ns + ~7.7ns/row
        per call; the ramp gets the first sub-gather's DMA flowing ~1.6us
        after the lib lands while bigger middle calls amortize the fixed
        cost and keep descriptor supply above the ~350GB/s DMA drain rate.
  SP  : stores even sub-blocks (1st HWDGE ring)

The gathered row i lands at SBUF partition i%128, col i//128; stores use a
matching strided DRAM view (output y [128, 16, 512]; host transposes back).
Index tile partitions 32..127 are memset to 0 (the queue-0 gather ucode
only reads partitions 0..31).
"""

import sys

import numpy as np

try:
    import concourse  # noqa: F401
except ImportError:  # pragma: no cover
    sys.path.insert(0, "/opt/trn_rl_repo")

N_CORES = 8
B_FULL = 16384
D = 4096
ROUTES = 8
RW = D // ROUTES
B_SH = B_FULL // N_CORES  # 2048
NJ = B_SH // 128  # 16 output cols
# sub-gather sizes in 128-row cols: ramp up (amortize the ~1us fixed
# ucode cost per call) and back down (short store tail)
JSIZES = [1, 2, 3, 4, 3, 2, 1]
JBOUNDS = [sum(JSIZES[:i]) for i in range(len(JSIZES) + 1)]  # [0,1,3,6,10,13,15,16]
NG = len(JSIZES)
KB = 4  # k-blocks in the [32, KB, 16, 8] routing layout

_prog_cache = {}


def _build_program():
    import concourse.bacc as bacc
    import concourse.mybir as mybir
    from concourse.library_config import mlp
    from contextlib import ExitStack

    f32 = mybir.dt.float32
    i32 = mybir.dt.int32
    i16 = mybir.dt.int16
    Alu = mybir.AluOpType
    Axis = mybir.AxisListType

    nc = bacc.Bacc("TRN2", target_bir_lowering=False, debug=False, num_devices=N_CORES)
    x = nc.dram_tensor("x", [B_SH, D], f32, kind="ExternalInput")
    rt = nc.dram_tensor("rt", [B_SH, ROUTES], f32, kind="ExternalInput")
    wt = nc.dram_tensor("wt", [32, KB, 16, ROUTES], f32, kind="ExternalInput")
    # partition-major output: y[p, j, :] holds row j*128+p; the host
    # transposes back. Keeps every store descriptor 4KB-contiguous.
    y = nc.dram_tensor("y", [128, NJ, RW], f32, kind="ExternalOutput")

    x_rows = x.ap().rearrange("b (r w) -> (b r) w", r=ROUTES)
    # row = 512k + 16p + u
    rt_n = rt.ap().rearrange("(k p u) r -> p k u r", k=KB, p=32, u=16)
    y_pjw = y.ap()

    with (
        ExitStack() as ctx,
        nc.sbuf_tensor("r_t", [32, KB, 16, ROUTES], f32) as r_t,
        nc.sbuf_tensor("wt_t", [32, KB, 16, ROUTES], f32) as wt_t,
        nc.sbuf_tensor("mx", [32, KB, 16], f32) as mx,
        nc.sbuf_tensor("eq", [32, KB, 16, ROUTES], f32) as eq,
        nc.sbuf_tensor("idf", [32, KB * 16], f32) as idf,
        nc.sbuf_tensor("idfi", [32, KB * 16], i32) as idfi,
        nc.sbuf_tensor("idx16", [128, 128], i16) as idx16,
        nc.sbuf_tensor("g_t", [128, NJ, RW], f32) as g_t,
        nc.Block(no_gpsimd_drain=True) as block,
    ):
        s_rt = ctx.enter_context(nc.semaphore("s_rt"))
        s_wt = ctx.enter_context(nc.semaphore("s_wt"))
        s_v = ctx.enter_context(nc.semaphore("s_v"))
        s_g = [ctx.enter_context(nc.semaphore(f"s_g{k}")) for k in range(NG)]
        s_y = ctx.enter_context(nc.semaphore("s_y"))

        @block.scalar
        def _(act):
            act.dma_start(r_t[:], rt_n).then_inc(s_rt, 16)
            act.dma_start(wt_t[:], wt.ap()).then_inc(s_wt, 16)
            for k in range(1, NG, 2):
                js = slice(JBOUNDS[k], JBOUNDS[k + 1])
                act.wait_ge(s_g[k], 16)
                act.dma_start(y_pjw[:, js, :], g_t[:, js, :]).then_inc(s_y, 16)

        @block.vector
        def _(dve):
            dve.memset(idx16[:], 0)
            dve.wait_ge(s_rt, 16)
            dve.tensor_reduce(mx[:], r_t[:], Axis.X, Alu.max)
            dve.tensor_tensor(
                eq[:],
                r_t[:],
                mx[:].unsqueeze(3).broadcast_to([32, KB, 16, ROUTES]),
                Alu.is_equal,
            )
            dve.wait_ge(s_wt, 16)
            dve.tensor_tensor(eq[:], eq[:], wt_t[:], Alu.mult)
            dve.tensor_reduce(idf[:], eq[:], Axis.X, Alu.add)
            dve.tensor_copy(idfi[:], idf[:])
            # low int16 halves, doubled along a 0-stride dim, then a 32x32
            # block transpose: idx16[q, 32k+16d+u] = idx(16c + q%16).
            t_in = (
                idfi.ap()
                .bitcast(i16)
                .rearrange("q (k u two) -> q k u two", k=KB, u=16, two=2)[:, :, :, 0]
                .unsqueeze(2)
                .broadcast_to([32, KB, 2, 16])
            )
            t_out = idx16.ap()[0:32, :].rearrange("q (k d u) -> q k d u", k=KB, d=2, u=16)
            dve.transpose(t_out, t_in).then_inc(s_v, 1)

        @block.gpsimd
        def _(pool):
            pool.load_library(mlp)
            pool.wait_ge(s_v, 1)
            for k, (j0, j1) in enumerate(zip(JBOUNDS, JBOUNDS[1:])):
                js = slice(j0, j1)
                cs = slice(8 * j0, 8 * j1)
                rows = 128 * (j1 - j0)
                pool.dma_gather(
                    g_t[:, js, :],
                    x_rows,
                    idx16[:, cs],
                    rows,
                    rows,
                    RW,
                    single_packet=False,
                ).then_inc(s_g[k], 16)

        @block.sync
        def _(sp):
            for k in range(0, NG, 2):
                js = slice(JBOUNDS[k], JBOUNDS[k + 1])
                sp.wait_ge(s_g[k], 16)
                sp.dma_start(y_pjw[:, js, :], g_t[:, js, :]).then_inc(s_y, 16)
            sp.wait_ge(s_y, 16 * NG)

    nc.compile()
    return nc


def _get_program():
    if "p" not in _prog_cache:
        _prog_cache["p"] = _build_program()
    return _prog_cache["p"]


def _weights():
    # wt[p, k, u, r] = r + 8*row, row = 512k + 16p + u
    p = np.arange(32, dtype=np.float32)[:, None, None, None]
    k = np.arange(KB, dtype=np.float32)[None, :, None, None]
    u = np.arange(16, dtype=np.float32)[None, None, :, None]
    r = np.arange(ROUTES, dtype=np.float32)[None, None, None, :]
    return np.ascontiguousarray(r + 8.0 * (512.0 * k + 16.0 * p + u), dtype=np.float32)


def _in_maps(inputs, routing_inputs):
    wt = _weights()
    return [
        {
            "x": inputs[c * B_SH : (c + 1) * B_SH],
            "rt": routing_inputs[c * B_SH : (c + 1) * B_SH],
            "wt": wt,
        }
        for c in range(N_CORES)
    ]


def kernel(inputs: np.ndarray, routing_inputs: np.ndarray) -> np.ndarray:
    from concourse.bass_utils import run_bass_kernel_spmd

    inputs = np.ascontiguousarray(inputs, dtype=np.float32)
    routing_inputs = np.ascontiguousarray(routing_inputs, dtype=np.float32)
    nc = _get_program()
    in_maps = _in_maps(inputs, routing_inputs)
    res = None
    for attempt in range(3):
        try:
            res = run_bass_kernel_spmd(nc, in_maps, core_ids=list(range(N_CORES)))
            break
        except Exception:  # transient NRT_EXEC_UNIT_UNRECOVERABLE flakes
            if attempt == 2:
                raise
            import time

            time.sleep(2.0)
    return np.concatenate(
        [
            res.results[c]["y"].transpose(1, 0, 2).reshape(B_SH, RW)
            for c in range(N_CORES)
        ],
        axis=0,
    )
